# revision 1
# baseline (speedup 1.0000x reference)
"""Guided filter on 8 Trainium2 NeuronCores — pure data parallel (1 image/core).

Per-image pipeline (B=8, C=3, H=W=512, r=15, eps=1e-6):
  gray = luma(guide);  box() = 31x31 zero-padded box mean (divide by 961)
  mean_I=box(gray), corr_I=box(gray^2), mean_p=box(p), corr_Ip=box(gray*p)
  var=corr_I-mean_I^2; a=cov/(var+eps); b=mean_p-a*mean_I
  out = box(a)*gray + box(b)

Box filter is separable:
  W-pass: ONE DVE tensor_tensor_scan per plane over the flattened padded
          layout [128, 4*560(+48)]: state=(x[s+31]+state)-x[s], with each
          row-block stored as [32 zeros | 512 data | 16 zeros]. The window
          telescopes so block boundaries come out exact: out[560b+t] is the
          31-wide box sum at column t of block b.
  H-pass: banded matmul T@Y on PE in float32r (T = 0/1 band matrix as three
          128x128 blocks D/U/L, fed as extra ExternalInputs), accumulating
          into a [128,4,512] PSUM plane tile (4 banks).
All 1/961 normalizations are folded into downstream STT scalar slots:
  mIp=31*mean_I, mpp=31*mean_p, vv=961*(var+eps), covR=961*cov, bR'=31*b.
"""
import sys

sys.path.insert(0, "/opt/trn_rl_repo")

import numpy as np
import concourse.bass as bass
import concourse.bacc as bacc
import concourse.mybir as mybir
import concourse.tile as tile
from concourse import bass_utils
from contextlib import ExitStack

F32 = mybir.dt.float32
F32R = mybir.dt.float32r
ALU = mybir.AluOpType

R = 15
KW = 31
H = W = 512
NB = 4
PADL, PADR = 32, 16
PW = PADL + W + PADR       # 560 per-block padded width
TAIL = 48                  # extra zero tail so shifted scan views stay in-bounds
FLATW = NB * PW            # 2240
FULLW = FLATW + TAIL       # 2288
K2 = 1.0 / (KW * KW)
S31 = 1.0 / KW
EPS = 1e-6
E961 = (KW * KW) * EPS
WR, WG, WB = 0.299, 0.587, 0.114


def _band_blocks():
    idx = np.arange(128)
    M = idx[None, :] - idx[:, None]            # M[k,r] = r-k
    D = (np.abs(M) <= R).astype(np.float32)
    U = (M >= 128 - R).astype(np.float32)      # block j=i+1 -> i
    L = (M <= -(128 - R)).astype(np.float32)   # block j=i-1 -> i
    return D, U, L


def _build(nc):
    guide_d = nc.dram_tensor("guide", [3, H, W], F32, kind="ExternalInput").ap()
    input_d = nc.dram_tensor("input", [3, H, W], F32, kind="ExternalInput").ap()
    D_d = nc.dram_tensor("Dm", [128, 128], F32R, kind="ExternalInput").ap()
    U_d = nc.dram_tensor("Um", [128, 128], F32R, kind="ExternalInput").ap()
    L_d = nc.dram_tensor("Lm", [128, 128], F32R, kind="ExternalInput").ap()
    out_d = nc.dram_tensor("out", [3, H, W], F32, kind="ExternalOutput").ap()

    def plane(dram, c):
        return dram[c].rearrange("(b p) w -> p b w", p=128)

    with tile.TileContext(nc) as tc, ExitStack() as ctx:
        cpool = ctx.enter_context(tc.tile_pool(name="consts", bufs=1))
        pads = ctx.enter_context(tc.tile_pool(name="pads", bufs=1))
        scano = ctx.enter_context(tc.tile_pool(name="scano", bufs=4))
        rndp = ctx.enter_context(tc.tile_pool(name="rndp", bufs=4))
        pers = ctx.enter_context(tc.tile_pool(name="pers", bufs=1))
        mppp = ctx.enter_context(tc.tile_pool(name="mppp", bufs=2))
        misc = ctx.enter_context(tc.tile_pool(name="misc", bufs=4))
        outp = ctx.enter_context(tc.tile_pool(name="outp", bufs=2))
        psum = ctx.enter_context(tc.tile_pool(name="psum", bufs=2, space="PSUM"))

        Dt = cpool.tile([128, 128], F32R, tag="D", name="Dt")
        Ut = cpool.tile([128, 128], F32R, tag="U", name="Ut")
        Lt = cpool.tile([128, 128], F32R, tag="L", name="Lt")
        nc.sync.dma_start(Dt[:], D_d)
        nc.sync.dma_start(Ut[:], U_d)
        nc.sync.dma_start(Lt[:], L_d)

        # ---- persistent padded plane buffers ----
        # flat layout [128, FULLW]; block b data at [560b+32, 560b+544)
        def padded(tag, zero_pads):
            t = pads.tile([128, FULLW], F32, tag=tag, name=tag)
            if zero_pads:
                tb = t[:, 0:FLATW].rearrange("p (b w) -> p b w", b=NB)
                nc.gpsimd.memset(tb[:, :, 0:PADL], 0.0)
                nc.gpsimd.memset(tb[:, :, PADL + W:PW], 0.0)
            nc.gpsimd.memset(t[:, FLATW:FULLW], 0.0)
            return t

        def data_view(t):
            """[128, NB, W] strided data view of a padded buffer"""
            return t[:, 0:FLATW].rearrange("p (b w) -> p b w", b=NB)[
                :, :, PADL:PADL + W]

        gray_b = padded("gray", True)
        gray2_b = padded("gray2", False)   # full-width writes keep pads zero
        cb1 = [padded(f"c1_{c}", True) for c in range(3)]   # p_c then a_c
        cb2 = [padded(f"c2_{c}", False) for c in range(3)]  # gp_c then bR_c

        for c in range(3):
            nc.sync.dma_start(data_view(cb1[c]), plane(input_d, c))

        # ---- gray ----
        gtiles = [misc.tile([128, NB, W], F32, tag="m", name=f"gt{c}")
                  for c in range(3)]
        for c in range(3):
            nc.sync.dma_start(gtiles[c][:], plane(guide_d, c))
        s1 = misc.tile([128, NB, W], F32, tag="m", name="s1")
        nc.vector.scalar_tensor_tensor(
            s1[:], gtiles[0][:], WR / WB, gtiles[2][:], ALU.mult, ALU.add)
        s2 = misc.tile([128, NB, W], F32, tag="m", name="s2")
        nc.vector.scalar_tensor_tensor(
            s2[:], gtiles[1][:], WG / WB, s1[:], ALU.mult, ALU.add)
        nc.scalar.mul(data_view(gray_b), s2[:], WB)

        # gray^2: full width (incl. tail) keeps pads zero
        nc.scalar.square(gray2_b[:], gray_b[:])

        # ---- box building blocks ----
        HB = NB // 2
        HW_ = HB * PW  # 1120: half-plane scan width

        def wscan(src):
            """two half-plane scans; any 560-aligned split carries zero
            state (the window telescopes to 0 inside each block's pads)"""
            halves = []
            for h in range(2):
                so = scano.tile([128, HW_], F32, tag="so", name=f"so{h}")
                off = h * HW_
                nc.vector.tensor_tensor_scan(
                    so[:],
                    src[:, off + KW:off + KW + HW_],   # x[s+31]
                    src[:, off:off + HW_],             # x[s]
                    0.0, ALU.add, ALU.subtract)
                halves.append(
                    so[:].rearrange("p (b w) -> p b w", b=HB)[:, :, 16:16 + W])
            return halves

        def round_f32r(halves, on_act):
            rnds = []
            for h, sov in enumerate(halves):
                rt = rndp.tile([128, HB, W], F32R, tag="rnd", name=f"rnd{h}")
                if on_act:
                    nc.scalar.copy(rt[:], sov)
                else:
                    nc.vector.tensor_copy(rt[:], sov)
                rnds.append(rt)
            return rnds

        def hpass(rnds):
            """banded matmul -> PSUM plane tile [128, NB, W] (4 banks)"""
            def rv(j):
                return rnds[j // HB][:, j % HB, :]
            ps = psum.tile([128, NB, W], F32, name="ps")
            for i in range(NB):
                mms = [(Dt, i)]
                if i + 1 < NB:
                    mms.append((Ut, i + 1))
                if i - 1 >= 0:
                    mms.append((Lt, i - 1))
                for n, (lhsT, j) in enumerate(mms):
                    nc.tensor.matmul(ps[:, i, :], lhsT[:], rv(j),
                                     start=(n == 0), stop=(n == len(mms) - 1))
            return ps

        def box_raw(src, on_act):
            return hpass(round_f32r(wscan(src), on_act))

        # ---- shared chain ----
        ps_mi = box_raw(gray_b, on_act=False)
        mIp = pers.tile([128, NB, W], F32, tag="mIp", name="mIp")
        nc.scalar.mul(mIp[:], ps_mi[:], S31)              # 31*mean_I
        ps_ci = box_raw(gray2_b, on_act=True)
        sq = misc.tile([128, NB, W], F32, tag="m", name="sq")
        nc.scalar.square(sq[:], mIp[:])                   # 961*mean_I^2
        vv = misc.tile([128, NB, W], F32, tag="m", name="vv")
        # vv = (sq*-1 + E961) + rawCI = 961*(var+eps)
        nc.vector.affine_then_add(vv[:], sq[:], ps_ci[:], -1.0, E961)
        rr = pers.tile([128, NB, W], F32, tag="rr", name="rr")
        nc.vector.reciprocal_approx_fast(rr[:], vv[:])

        # ---- per-channel chains ----
        for c in range(3):
            p_b, gp_b = cb1[c], cb2[c]
            # gp = gray*p over the full buffer (keeps pads zero)
            nc.gpsimd.tensor_mul(gp_b[:], gray_b[:], p_b[:])

            ps_mp = box_raw(p_b, on_act=False)
            mpp = mppp.tile([128, NB, W], F32, tag="mpp", name="mpp")
            nc.scalar.mul(mpp[:], ps_mp[:], S31)          # 31*mean_p
            ps_ip = box_raw(gp_b, on_act=True)

            u2 = misc.tile([128, NB, W], F32, tag="m", name="u2")
            nc.gpsimd.tensor_mul(u2[:], mIp[:], mpp[:])   # 961*mI*mp
            covR = misc.tile([128, NB, W], F32, tag="m", name="covR")
            nc.vector.scalar_tensor_tensor(
                covR[:], u2[:], -1.0, ps_ip[:], ALU.mult, ALU.add)
            # a = covR*rr -> p buffer's data region (p is dead)
            a_v = data_view(p_b)
            nc.vector.tensor_mul(a_v, covR[:], rr[:])
            # v' = a*mIp ; bR' = mpp - v' -> gp buffer (dead)
            vpr = misc.tile([128, NB, W], F32, tag="m", name="vpr")
            nc.gpsimd.tensor_mul(vpr[:], a_v, mIp[:])
            nc.gpsimd.tensor_sub(data_view(gp_b), mpp[:], vpr[:])

            ps_ma = box_raw(p_b, on_act=False)     # box(a)
            ps_mb = box_raw(gp_b, on_act=True)     # box(31*b)

            w_v = misc.tile([128, NB, W], F32, tag="m", name="w_v")
            # w = (rawMa*K2)*gray
            nc.vector.scalar_tensor_tensor(
                w_v[:], ps_ma[:], K2, data_view(gray_b), ALU.mult, ALU.mult)
            ot = outp.tile([128, NB, W], F32, tag="out", name="ot")
            # out = (rawMb*(K2/31)) + w
            nc.vector.scalar_tensor_tensor(
                ot[:], ps_mb[:], K2 * S31, w_v[:], ALU.mult, ALU.add)
            nc.sync.dma_start(plane(out_d, c), ot[:])

    nc.compile()
    return nc


_NC_CACHE = None


def _get_nc():
    global _NC_CACHE
    if _NC_CACHE is None:
        nc = bacc.Bacc("TRN2", target_bir_lowering=False, debug=False)
        _build(nc)
        _NC_CACHE = nc
    return _NC_CACHE


def kernel(**inputs):
    guide = np.ascontiguousarray(inputs["guide"], dtype=np.float32)
    inp = np.ascontiguousarray(inputs["input"], dtype=np.float32)
    B = guide.shape[0]
    assert guide.shape == (8, 3, H, W) and inp.shape == (8, 3, H, W)
    D, U, L = _band_blocks()
    nc = _get_nc()
    in_maps = [
        {"guide": guide[i], "input": inp[i], "Dm": D, "Um": U, "Lm": L}
        for i in range(B)
    ]
    res = bass_utils.run_bass_kernel_spmd(nc, in_maps, core_ids=list(range(B)))
    return np.stack([res.results[i]["out"] for i in range(B)], axis=0)



# revision 2
# speedup vs baseline: 1.0068x; 1.0068x over previous
"""Guided filter on 8 NeuronCores — fp16 pipeline v10.

Hardware-legal op placement (verified against the walrus BIR verifier):
  - tensor_tensor_scan and scalar_tensor_tensor run ONLY on DVE
  - gpsimd (Pool) cannot touch PSUM; it gets plain TT/TSP/memset/SWDGE
  - matmul PSUM writes are limited to one bank (free <= 512)

Structure per box: W-pass = one full-plane DVE scan over the padded
fp16 plane [128, 4*560(+48)] -> fp16 scan tile; H-pass = banded
matmuls (fp16, free=512 per PSUM bank), with extra identity-matmul
folds: In*u2 turns the gp-box PSUM into 961*cov directly, Im*w adds
the mean_a*gray term into the b-box PSUM.

Engines: DVE scans + critical elementwise (2x fp16 TT); Pool u2/vpr/bR
TTs + cast-DMA descgen + pad memsets; Act all PSUM extracts (with
constant folds) and squares; PE band matmuls + warmup ramp chain.

Scales: mI=31*mean_I, mp=31*mean_p, vv=961*(var+eps), covR=961*cov,
bR=31*b. b-pass matrices carry 32/(961*31), Im=32*I, In=-I; the final
Act copy scales by 1/32.
"""
import sys

sys.path.insert(0, "/opt/trn_rl_repo")

import numpy as np
import concourse.bass as bass
import concourse.bacc as bacc
import concourse.mybir as mybir
import concourse.tile as tile
from concourse import bass_utils
from contextlib import ExitStack

F32 = mybir.dt.float32
F16 = mybir.dt.float16
ALU = mybir.AluOpType

R = 15
KW = 31
H = W = 512
NB = 4
PADL, PADR = 32, 16
PW = PADL + W + PADR       # 560
TAIL = 48
FLATW = NB * PW            # 2240
FULLW = FLATW + TAIL       # 2288
K2 = 1.0 / (KW * KW)
S31 = 1.0 / KW
EPS = 1e-6
E961 = (KW * KW) * EPS
WR, WG, WB = 0.299, 0.587, 0.114
BSC = 32.0
SO_N = 6
WARM_N = 40


def _band_blocks():
    idx = np.arange(128)
    M = idx[None, :] - idx[:, None]
    D = (np.abs(M) <= R).astype(np.float16)
    U = (M >= 128 - R).astype(np.float16)
    L = (M <= -(128 - R)).astype(np.float16)
    s = np.float16(BSC * K2 * S31)
    return {"Im": (BSC * np.eye(128)).astype(np.float16),
            "In": (-np.eye(128)).astype(np.float16),
            "Dm": D, "Um": U, "Lm": L,
            "Ds": (D * s).astype(np.float16),
            "Us": (U * s).astype(np.float16),
            "Ls": (L * s).astype(np.float16)}


MATS_KEY = ("Im", "In", "Dm", "Um", "Lm", "Ds", "Us", "Ls")


def _build(nc):
    guide_d = nc.dram_tensor("guide", [3, H, W], F32, kind="ExternalInput").ap()
    input_d = nc.dram_tensor("input", [3, H, W], F32, kind="ExternalInput").ap()
    mats_all_d = nc.dram_tensor("Ms", [8, 128, 128], F16,
                                kind="ExternalInput").ap()
    out_d = nc.dram_tensor("out", [3, H, W], F32, kind="ExternalOutput").ap()

    def plane(dram, c):
        return dram[c].rearrange("(b p) w -> p b w", p=128)

    with tile.TileContext(nc) as tc, ExitStack() as ctx:
        cpool = ctx.enter_context(tc.tile_pool(name="consts", bufs=1))
        pads = ctx.enter_context(tc.tile_pool(name="pads", bufs=1))
        sop = ctx.enter_context(tc.tile_pool(name="sop", bufs=1))
        gt = ctx.enter_context(tc.tile_pool(name="gt", bufs=1))
        pers = ctx.enter_context(tc.tile_pool(name="pers", bufs=1))
        mpq = ctx.enter_context(tc.tile_pool(name="mpq", bufs=3))
        cvq = ctx.enter_context(tc.tile_pool(name="cvq", bufs=2))
        vwq = ctx.enter_context(tc.tile_pool(name="vwq", bufs=2))
        outp = ctx.enter_context(tc.tile_pool(name="outp", bufs=2))
        psum = ctx.enter_context(tc.tile_pool(name="psum", bufs=4, space="PSUM"))

        mats_t = cpool.tile([128, 8, 128], F16, tag="Ms", name="Ms")
        nc.sync.dma_start(mats_t[:], mats_all_d.rearrange("k p c -> p k c"))
        mats = {k: mats_t[:, i, :] for i, k in enumerate(MATS_KEY)}

        def padded(tag):
            return pads.tile([128, FULLW], F16, tag=tag, name=tag)

        def data_view(t):
            return t[:, 0:FLATW].rearrange("p (b w) -> p b w", b=NB)[
                :, :, PADL:PADL + W]

        def data_view3(t, c):
            return t[:, c, 0:FLATW].rearrange("p (b w) -> p b w", b=NB)[
                :, :, PADL:PADL + W]

        g16 = padded("g16")
        g2b = padded("g2b")
        pball = pads.tile([128, 3, FULLW], F16, tag="pball", name="pball")
        gpb = [padded(f"gp{c}") for c in range(3)]

        sot = [sop.tile([128, FLATW], F16, tag=f"so{i}", name=f"so{i}")
               for i in range(SO_N)]

        # ---- PE warm-up chain on Im (lands first in the mats blob) ----
        ps_warm = psum.tile([128, 2, W], F32, name="ps")
        for i in range(WARM_N):
            nc.tensor.matmul(ps_warm[:, 0, 0:128], mats["Im"], mats["Im"],
                             start=(i == 0), stop=(i == WARM_N - 1))

        # ---- casting input DMAs (SWDGE); Pool stream order matters ----
        gtiles = [gt.tile([128, NB, W], F16, tag=f"g{c}", name=f"g{c}")
                  for c in range(3)]
        nc.gpsimd.dma_start(data_view3(pball, 0), plane(input_d, 0))
        pb4 = pball[:, :, 0:FLATW].rearrange("p c (b w) -> p c b w", b=NB)
        nc.gpsimd.memset(pb4[:, :, :, 0:PADL], 0.0)
        nc.gpsimd.memset(pb4[:, :, :, PADL + W:PW], 0.0)
        nc.gpsimd.memset(pball[:, :, FLATW:FULLW], 0.0)
        for c in range(3):
            nc.gpsimd.dma_start(gtiles[c][:], plane(guide_d, c))
        nc.gpsimd.dma_start(data_view3(pball, 1), plane(input_d, 1))
        nc.gpsimd.dma_start(data_view3(pball, 2), plane(input_d, 2))
        gb4 = g16[:, 0:FLATW].rearrange("p (b w) -> p b w", b=NB)
        nc.gpsimd.memset(gb4[:, :, 0:PADL], 0.0)
        nc.gpsimd.memset(gb4[:, :, PADL + W:PW], 0.0)
        nc.gpsimd.memset(g16[:, FLATW:FULLW], 0.0)
        for t in (g2b, gpb[0], gpb[1], gpb[2]):
            nc.gpsimd.memset(t[:, FLATW:FULLW], 0.0)

        # ---- luma on DVE (TSP prescale at 4x + two 2x adds) ----
        for c, wc in ((0, WR), (1, WG), (2, WB)):
            nc.vector.tensor_scalar_mul(gtiles[c][:], gtiles[c][:], wc)
        nc.vector.tensor_add(gtiles[0][:], gtiles[0][:], gtiles[1][:])
        nc.vector.tensor_add(data_view(g16), gtiles[0][:], gtiles[2][:])

        # g^2 full-width on Act (pads stay zero)
        nc.scalar.square(g2b[:, 0:FLATW], g16[:, 0:FLATW])

        # ---- box building blocks ----
        so_ctr = [0]

        def wscan(src):
            so = sot[so_ctr[0] % SO_N]
            so_ctr[0] += 1
            nc.vector.tensor_tensor_scan(
                so[:], src[:, KW:KW + FLATW], src[:, 0:FLATW],
                0.0, ALU.add, ALU.subtract)
            return so

        def hpass(so, scaled=False, w_pre=None, post_sub=None):
            """two half-plane PSUM groups [128, 2, 512] (2 banks, 4 slots)"""
            Dm, Um, Lm = (mats["Ds"], mats["Us"], mats["Ls"]) if scaled else (
                mats["Dm"], mats["Um"], mats["Lm"])
            rvb = so[:].rearrange("p (b w) -> p b w", b=NB)

            def rv(j):
                return rvb[:, j, 16:16 + W]

            halves = []
            for h in range(2):
                ps = psum.tile([128, 2, W], F32, name="ps")
                for k in range(2):
                    i = 2 * h + k
                    mms = []
                    if w_pre is not None:
                        mms.append((mats["Im"], w_pre[:, i, :]))
                    mms.append((Dm, rv(i)))
                    if i + 1 < NB:
                        mms.append((Um, rv(i + 1)))
                    if i - 1 >= 0:
                        mms.append((Lm, rv(i - 1)))
                    if post_sub is not None:
                        mms.append((mats["In"], post_sub[:, i, :]))
                    for n, (lhsT, rhs) in enumerate(mms):
                        nc.tensor.matmul(ps[:, k, :], lhsT, rhs,
                                         start=(n == 0),
                                         stop=(n == len(mms) - 1))
                halves.append(ps)
            return halves

        def extract(dst, halves, fn):
            """dst: [128, NB, W]-shaped tile; apply fn per half"""
            for h, ps in enumerate(halves):
                fn(dst[:, 2 * h:2 * h + 2, :], ps[:])

        # ---- round 1 ----
        so_p0 = wscan(pball[:, 0, :])                  # slot 0
        hs_p0 = hpass(so_p0)
        mp16 = [mpq.tile([128, NB, W], F16, tag="mp", name=f"mp{c}")
                for c in range(3)]
        extract(mp16[0][:], hs_p0, lambda d, s_: nc.scalar.mul(d, s_, S31))

        so_g = wscan(g16)                              # slot 1
        hs_g = hpass(so_g)
        mI = pers.tile([128, NB, W], F16, tag="mI", name="mI")
        extract(mI[:], hs_g, lambda d, s_: nc.scalar.mul(d, s_, S31))

        so_g2 = wscan(g2b)                             # slot 2
        hs_g2 = hpass(so_g2)

        sq = pers.tile([128, NB, W], F16, tag="sq", name="sq")
        nc.scalar.square(sq[:], mI[:])
        vv = pers.tile([128, NB, W], F32, tag="vv", name="vv")
        for h, ps in enumerate(hs_g2):
            nc.vector.affine_then_add(vv[:, 2 * h:2 * h + 2, :],
                                      sq[:, 2 * h:2 * h + 2, :],
                                      ps[:], -1.0, E961)
        rr = pers.tile([128, NB, W], F32, tag="rr", name="rr")
        nc.vector.reciprocal_approx_fast(rr[:], vv[:])

        cvl = [cvq.tile([128, NB, W], F16, tag="cv", name=f"cv{c}")
               for c in range(3)]
        u2q = [cvq.tile([128, NB, W], F16, tag="u2", name=f"u2_{c}")
               for c in range(3)]

        def gp_box(c):
            # gp = g*p full-width on DVE (pads 0*0=0)
            nc.vector.tensor_mul(gpb[c][:, 0:FLATW], g16[:, 0:FLATW],
                                 pball[:, c, 0:FLATW])
            so_gp = wscan(gpb[c])
            nc.vector.tensor_mul(u2q[c][:], mI[:], mp16[c][:])
            hs = hpass(so_gp, post_sub=u2q[c])         # 961*cov in PSUM
            extract(cvl[c][:], hs, lambda d, s_: nc.scalar.copy(d, s_))
            return hs

        def p_box(c):
            so_p = wscan(pball[:, c, :])
            hs = hpass(so_p)
            extract(mp16[c][:], hs, lambda d, s_: nc.scalar.mul(d, s_, S31))

        def ab_planes(c):
            a_v = data_view3(pball, c)
            nc.vector.tensor_mul(a_v, cvl[c][:], rr[:])      # a (rr fp32)
            vpr = vwq.tile([128, NB, W], F16, tag="vpr", name=f"vpr{c}")
            nc.vector.tensor_mul(vpr[:], a_v, mI[:])         # a*mI
            nc.gpsimd.tensor_sub(data_view(gpb[c]), mp16[c][:], vpr[:])

        ps_gp0 = gp_box(0)                             # slot 3, psum 0
        p_box(1)                                       # slot 4, psum 1
        ab_planes(0)
        ps_gp1 = gp_box(1)                             # slot 5, psum 0
        p_box(2)                                       # slot 0, psum 1
        ab_planes(1)
        ps_gp2 = gp_box(2)                             # slot 1, psum 0
        ab_planes(2)

        # ---- round 2: scans first, then mm/extract phase ----
        soa = [None] * 3
        sob = [None] * 3
        for c in range(3):
            soa[c] = wscan(pball[:, c, :])       # slots 2, 3, 4
        for c in range(3):
            sob[c] = wscan(gpb[c])               # slots 5, 0, 1

        wl = [None] * 3
        for c in range(3):
            hsa = hpass(soa[c])
            ea = vwq.tile([128, NB, W], F16, tag="ea", name=f"ea{c}")
            extract(ea[:], hsa, lambda d, s_: nc.scalar.mul(d, s_, K2))
            wl[c] = vwq.tile([128, NB, W], F16, tag="w", name=f"w{c}")
            nc.vector.tensor_mul(wl[c][:], ea[:], data_view(g16))

        for c in range(3):
            hsb = hpass(sob[c], scaled=True, w_pre=wl[c])
            o32 = outp.tile([128, NB, W], F32, tag="o", name=f"o{c}")
            extract(o32[:], hsb,
                    lambda d, s_: nc.scalar.mul(d, s_, 1.0 / BSC))
            nc.sync.dma_start(plane(out_d, c), o32[:])

    nc.compile()
    return nc


_NC_CACHE = None


def _get_nc():
    global _NC_CACHE
    if _NC_CACHE is None:
        nc = bacc.Bacc("TRN2", target_bir_lowering=False, debug=False)
        _build(nc)
        _NC_CACHE = nc
    return _NC_CACHE


def kernel(**inputs):
    guide = np.ascontiguousarray(inputs["guide"], dtype=np.float32)
    inp = np.ascontiguousarray(inputs["input"], dtype=np.float32)
    B = guide.shape[0]
    assert guide.shape == (8, 3, H, W) and inp.shape == (8, 3, H, W)
    mats = _band_blocks()
    ms = np.stack([mats[k] for k in MATS_KEY], axis=0)
    nc = _get_nc()
    in_maps = [
        {"guide": guide[i], "input": inp[i], "Ms": ms}
        for i in range(B)
    ]
    res = bass_utils.run_bass_kernel_spmd(nc, in_maps, core_ids=list(range(B)))
    return np.stack([res.results[i]["out"] for i in range(B)], axis=0)


# revision 3
# speedup vs baseline: 1.0831x; 1.0758x over previous
"""Guided filter on 8 NeuronCores — fp16 pipeline v10.

Hardware-legal op placement (verified against the walrus BIR verifier):
  - tensor_tensor_scan and scalar_tensor_tensor run ONLY on DVE
  - gpsimd (Pool) cannot touch PSUM; it gets plain TT/TSP/memset/SWDGE
  - matmul PSUM writes are limited to one bank (free <= 512)

Structure per box: W-pass = one full-plane DVE scan over the padded
fp16 plane [128, 4*560(+48)] -> fp16 scan tile; H-pass = banded
matmuls (fp16, free=512 per PSUM bank), with extra identity-matmul
folds: In*u2 turns the gp-box PSUM into 961*cov directly, Im*w adds
the mean_a*gray term into the b-box PSUM.

Engines: DVE scans + critical elementwise (2x fp16 TT); Pool u2/vpr/bR
TTs + cast-DMA descgen + pad memsets; Act all PSUM extracts (with
constant folds) and squares; PE band matmuls + warmup ramp chain.

Scales: mI=31*mean_I, mp=31*mean_p, vv=961*(var+eps), covR=961*cov,
bR=31*b. b-pass matrices carry 32/(961*31), Im=32*I, In=-I; the final
Act copy scales by 1/32.
"""
import sys

sys.path.insert(0, "/opt/trn_rl_repo")

import numpy as np
import concourse.bass as bass
import concourse.bacc as bacc
import concourse.mybir as mybir
import concourse.tile as tile
from concourse import bass_utils
from contextlib import ExitStack

F32 = mybir.dt.float32
F16 = mybir.dt.float16
ALU = mybir.AluOpType

R = 15
KW = 31
H = W = 512
NB = 4
PADL, PADR = 32, 16
PW = PADL + W + PADR       # 560
TAIL = 48
FLATW = NB * PW            # 2240
FULLW = FLATW + TAIL       # 2288
K2 = 1.0 / (KW * KW)
S31 = 1.0 / KW
EPS = 1e-6
E961 = (KW * KW) * EPS
WR, WG, WB = 0.299, 0.587, 0.114
BSC = 32.0
SO_N = 6
WARM_N = 40


def _band_blocks():
    idx = np.arange(128)
    M = idx[None, :] - idx[:, None]
    D = (np.abs(M) <= R).astype(np.float16)
    U = (M >= 128 - R).astype(np.float16)
    L = (M <= -(128 - R)).astype(np.float16)
    s = np.float16(BSC * K2 * S31)
    return {"Im": (BSC * np.eye(128)).astype(np.float16),
            "In": (-np.eye(128)).astype(np.float16),
            "Dm": D, "Um": U, "Lm": L,
            "Ds": (D * s).astype(np.float16),
            "Us": (U * s).astype(np.float16),
            "Ls": (L * s).astype(np.float16)}


MATS_KEY = ("Im", "In", "Dm", "Um", "Lm", "Ds", "Us", "Ls")


def _build(nc):
    guide_d = nc.dram_tensor("guide", [3, H, W], F32, kind="ExternalInput").ap()
    input_d = nc.dram_tensor("input", [3, H, W], F32, kind="ExternalInput").ap()
    mats_all_d = nc.dram_tensor("Ms", [8, 128, 128], F16,
                                kind="ExternalInput").ap()
    out_d = nc.dram_tensor("out", [3, H, W], F32, kind="ExternalOutput").ap()

    def plane(dram, c):
        return dram[c].rearrange("(b p) w -> p b w", p=128)

    with tile.TileContext(nc) as tc, ExitStack() as ctx:
        cpool = ctx.enter_context(tc.tile_pool(name="consts", bufs=1))
        pads = ctx.enter_context(tc.tile_pool(name="pads", bufs=1))
        sop = ctx.enter_context(tc.tile_pool(name="sop", bufs=1))
        gt = ctx.enter_context(tc.tile_pool(name="gt", bufs=1))
        pers = ctx.enter_context(tc.tile_pool(name="pers", bufs=1))
        mpq = ctx.enter_context(tc.tile_pool(name="mpq", bufs=3))
        cvq = ctx.enter_context(tc.tile_pool(name="cvq", bufs=2))
        vwq = ctx.enter_context(tc.tile_pool(name="vwq", bufs=2))
        outp = ctx.enter_context(tc.tile_pool(name="outp", bufs=2))
        psum = ctx.enter_context(tc.tile_pool(name="psum", bufs=4, space="PSUM"))

        mats_t = cpool.tile([128, 8, 128], F16, tag="Ms", name="Ms")
        nc.sync.dma_start(mats_t[:], mats_all_d.rearrange("k p c -> p k c"))
        mats = {k: mats_t[:, i, :] for i, k in enumerate(MATS_KEY)}

        def padded(tag):
            return pads.tile([128, FULLW], F16, tag=tag, name=tag)

        def data_view(t):
            return t[:, 0:FLATW].rearrange("p (b w) -> p b w", b=NB)[
                :, :, PADL:PADL + W]

        def data_view3(t, c):
            return t[:, c, 0:FLATW].rearrange("p (b w) -> p b w", b=NB)[
                :, :, PADL:PADL + W]

        g16 = padded("g16")
        g2b = padded("g2b")
        pball = pads.tile([128, 3, FULLW], F16, tag="pball", name="pball")
        gpb = [padded(f"gp{c}") for c in range(3)]

        sot = [sop.tile([128, FLATW], F16, tag=f"so{i}", name=f"so{i}")
               for i in range(SO_N)]

        # ---- PE warm-up chain on Im (lands first in the mats blob) ----
        ps_warm = psum.tile([128, 2, W], F32, name="ps")
        for i in range(WARM_N):
            nc.tensor.matmul(ps_warm[:, 0, 0:128], mats["Im"], mats["Im"],
                             start=(i == 0), stop=(i == WARM_N - 1))

        # ---- casting input DMAs (SWDGE); Pool stream order matters ----
        gtiles = [gt.tile([128, NB, W], F16, tag=f"g{c}", name=f"g{c}")
                  for c in range(3)]
        nc.gpsimd.dma_start(data_view3(pball, 0), plane(input_d, 0))
        pb4 = pball[:, :, 0:FLATW].rearrange("p c (b w) -> p c b w", b=NB)
        nc.gpsimd.memset(pb4[:, :, :, 0:PADL], 0.0)
        nc.gpsimd.memset(pb4[:, :, :, PADL + W:PW], 0.0)
        nc.gpsimd.memset(pball[:, :, FLATW:FULLW], 0.0)
        for c in range(3):
            nc.gpsimd.dma_start(gtiles[c][:], plane(guide_d, c))
        nc.gpsimd.dma_start(data_view3(pball, 1), plane(input_d, 1))
        nc.gpsimd.dma_start(data_view3(pball, 2), plane(input_d, 2))
        gb4 = g16[:, 0:FLATW].rearrange("p (b w) -> p b w", b=NB)
        nc.gpsimd.memset(gb4[:, :, 0:PADL], 0.0)
        nc.gpsimd.memset(gb4[:, :, PADL + W:PW], 0.0)
        nc.gpsimd.memset(g16[:, FLATW:FULLW], 0.0)
        for t in (g2b, gpb[0], gpb[1], gpb[2]):
            nc.gpsimd.memset(t[:, FLATW:FULLW], 0.0)

        # ---- luma on DVE (TSP prescale at 4x + two 2x adds) ----
        for c, wc in ((0, WR), (1, WG), (2, WB)):
            nc.vector.tensor_scalar_mul(gtiles[c][:], gtiles[c][:], wc)
        nc.vector.tensor_add(gtiles[0][:], gtiles[0][:], gtiles[1][:])
        nc.vector.tensor_add(data_view(g16), gtiles[0][:], gtiles[2][:])

        # g^2 full-width on Act (pads stay zero)
        nc.scalar.square(g2b[:, 0:FLATW], g16[:, 0:FLATW])

        # ---- box building blocks ----
        so_ctr = [0]

        def wscan(src):
            so = sot[so_ctr[0] % SO_N]
            so_ctr[0] += 1
            nc.vector.tensor_tensor_scan(
                so[:], src[:, KW:KW + FLATW], src[:, 0:FLATW],
                0.0, ALU.add, ALU.subtract)
            return so

        def hpass(so, scaled=False, w_pre=None, post_sub=None):
            """two half-plane PSUM groups [128, 2, 512] (2 banks, 4 slots)"""
            Dm, Um, Lm = (mats["Ds"], mats["Us"], mats["Ls"]) if scaled else (
                mats["Dm"], mats["Um"], mats["Lm"])
            rvb = so[:].rearrange("p (b w) -> p b w", b=NB)

            def rv(j):
                return rvb[:, j, 16:16 + W]

            halves = []
            for h in range(2):
                ps = psum.tile([128, 2, W], F32, name="ps")
                for k in range(2):
                    i = 2 * h + k
                    mms = []
                    if w_pre is not None:
                        mms.append((mats["Im"], w_pre[:, i, :]))
                    mms.append((Dm, rv(i)))
                    if i + 1 < NB:
                        mms.append((Um, rv(i + 1)))
                    if i - 1 >= 0:
                        mms.append((Lm, rv(i - 1)))
                    if post_sub is not None:
                        mms.append((mats["In"], post_sub[:, i, :]))
                    for n, (lhsT, rhs) in enumerate(mms):
                        nc.tensor.matmul(ps[:, k, :], lhsT, rhs,
                                         start=(n == 0),
                                         stop=(n == len(mms) - 1))
                halves.append(ps)
            return halves

        def extract(dst, halves, fn):
            """dst: [128, NB, W]-shaped tile; apply fn per half"""
            for h, ps in enumerate(halves):
                fn(dst[:, 2 * h:2 * h + 2, :], ps[:])

        # ---- round 1 ----
        mp16 = [mpq.tile([128, NB, W], F16, tag="mp", name=f"mp{c}")
                for c in range(3)]

        def p_box(c):
            so_p = wscan(pball[:, c, :])
            hs = hpass(so_p)
            extract(mp16[c][:], hs, lambda d, s_: nc.scalar.mul(d, s_, S31))

        p_box(0)                                       # slot 0

        p_box(1)                                       # slot 1
        so_g = wscan(g16)                              # slot 2
        hs_g = hpass(so_g)
        mI = pers.tile([128, NB, W], F16, tag="mI", name="mI")
        extract(mI[:], hs_g, lambda d, s_: nc.scalar.mul(d, s_, S31))

        so_g2 = wscan(g2b)                             # slot 2
        hs_g2 = hpass(so_g2)

        sq = pers.tile([128, NB, W], F16, tag="sq", name="sq")
        nc.vector.tensor_mul(sq[:], mI[:], mI[:])
        vv = pers.tile([128, NB, W], F32, tag="vv", name="vv")
        for h, ps in enumerate(hs_g2):
            nc.vector.affine_then_add(vv[:, 2 * h:2 * h + 2, :],
                                      sq[:, 2 * h:2 * h + 2, :],
                                      ps[:], -1.0, E961)
        rr = pers.tile([128, NB, W], F32, tag="rr", name="rr")
        nc.vector.reciprocal_approx_fast(rr[:], vv[:])

        cvl = [cvq.tile([128, NB, W], F16, tag="cv", name=f"cv{c}")
               for c in range(3)]
        u2q = [cvq.tile([128, NB, W], F16, tag="u2", name=f"u2_{c}")
               for c in range(3)]

        def gp_box(c):
            # gp = g*p full-width on DVE (pads 0*0=0)
            nc.vector.tensor_mul(gpb[c][:, 0:FLATW], g16[:, 0:FLATW],
                                 pball[:, c, 0:FLATW])
            so_gp = wscan(gpb[c])
            nc.vector.tensor_mul(u2q[c][:], mI[:], mp16[c][:])
            hs = hpass(so_gp, post_sub=u2q[c])         # 961*cov in PSUM
            extract(cvl[c][:], hs, lambda d, s_: nc.scalar.copy(d, s_))
            return hs

        def ab_planes(c):
            a_v = data_view3(pball, c)
            nc.vector.tensor_mul(a_v, cvl[c][:], rr[:])      # a (rr fp32)
            vpr = vwq.tile([128, NB, W], F16, tag="vpr", name=f"vpr{c}")
            nc.vector.tensor_mul(vpr[:], a_v, mI[:])         # a*mI
            nc.gpsimd.tensor_sub(data_view(gpb[c]), mp16[c][:], vpr[:])

        ps_gp0 = gp_box(0)
        ab_planes(0)
        ps_gp1 = gp_box(1)                             # slot 5, psum 0
        p_box(2)                                       # slot 0, psum 1
        ab_planes(1)
        ps_gp2 = gp_box(2)                             # slot 1, psum 0
        ab_planes(2)

        # ---- round 2: scans first, then mm/extract phase ----
        soa = [None] * 3
        sob = [None] * 3
        for c in range(3):
            soa[c] = wscan(pball[:, c, :])
            sob[c] = wscan(gpb[c])

        wl = [None] * 3
        for c in range(3):
            hsa = hpass(soa[c])
            ea = vwq.tile([128, NB, W], F16, tag="ea", name=f"ea{c}")
            extract(ea[:], hsa, lambda d, s_: nc.scalar.mul(d, s_, K2))
            wl[c] = vwq.tile([128, NB, W], F16, tag="w", name=f"w{c}")
            nc.vector.tensor_mul(wl[c][:], ea[:], data_view(g16))

        for c in range(3):
            hsb = hpass(sob[c], scaled=True, w_pre=wl[c])
            o32 = outp.tile([128, NB, W], F32, tag="o", name=f"o{c}")
            extract(o32[:], hsb,
                    lambda d, s_: nc.scalar.mul(d, s_, 1.0 / BSC))
            nc.sync.dma_start(plane(out_d, c), o32[:])

    nc.compile()
    return nc


_NC_CACHE = None


def _get_nc():
    global _NC_CACHE
    if _NC_CACHE is None:
        nc = bacc.Bacc("TRN2", target_bir_lowering=False, debug=False)
        _build(nc)
        _NC_CACHE = nc
    return _NC_CACHE


def kernel(**inputs):
    guide = np.ascontiguousarray(inputs["guide"], dtype=np.float32)
    inp = np.ascontiguousarray(inputs["input"], dtype=np.float32)
    B = guide.shape[0]
    assert guide.shape == (8, 3, H, W) and inp.shape == (8, 3, H, W)
    mats = _band_blocks()
    ms = np.stack([mats[k] for k in MATS_KEY], axis=0)
    nc = _get_nc()
    in_maps = [
        {"guide": guide[i], "input": inp[i], "Ms": ms}
        for i in range(B)
    ]
    res = bass_utils.run_bass_kernel_spmd(nc, in_maps, core_ids=list(range(B)))
    return np.stack([res.results[i]["out"] for i in range(B)], axis=0)


# revision 4
# speedup vs baseline: 1.1163x; 1.0307x over previous
"""Guided filter on 8 NeuronCores — fp16 pipeline v10.

Hardware-legal op placement (verified against the walrus BIR verifier):
  - tensor_tensor_scan and scalar_tensor_tensor run ONLY on DVE
  - gpsimd (Pool) cannot touch PSUM; it gets plain TT/TSP/memset/SWDGE
  - matmul PSUM writes are limited to one bank (free <= 512)

Structure per box: W-pass = one full-plane DVE scan over the padded
fp16 plane [128, 4*560(+48)] -> fp16 scan tile; H-pass = banded
matmuls (fp16, free=512 per PSUM bank), with extra identity-matmul
folds: In*u2 turns the gp-box PSUM into 961*cov directly, Im*w adds
the mean_a*gray term into the b-box PSUM.

Engines: DVE scans + critical elementwise (2x fp16 TT); Pool u2/vpr/bR
TTs + cast-DMA descgen + pad memsets; Act all PSUM extracts (with
constant folds) and squares; PE band matmuls + warmup ramp chain.

Scales: mI=31*mean_I, mp=31*mean_p, vv=961*(var+eps), covR=961*cov,
bR=31*b. b-pass matrices carry 32/(961*31), Im=32*I, In=-I; the final
Act copy scales by 1/32.
"""
import sys

sys.path.insert(0, "/opt/trn_rl_repo")

import numpy as np
import concourse.bass as bass
import concourse.bacc as bacc
import concourse.mybir as mybir
import concourse.tile as tile
from concourse import bass_utils
from contextlib import ExitStack

F32 = mybir.dt.float32
F16 = mybir.dt.float16
ALU = mybir.AluOpType

R = 15
KW = 31
H = W = 512
NB = 4
PADL, PADR = 32, 16
PW = PADL + W + PADR       # 560
TAIL = 48
FLATW = NB * PW            # 2240
FULLW = FLATW + TAIL       # 2288
K2 = 1.0 / (KW * KW)
S31 = 1.0 / KW
EPS = 1e-6
E961 = (KW * KW) * EPS
WR, WG, WB = 0.299, 0.587, 0.114
BSC = 32.0
SO_N = 6
WARM_N = 40


def _band_blocks():
    idx = np.arange(128)
    M = idx[None, :] - idx[:, None]
    D = (np.abs(M) <= R).astype(np.float16)
    U = (M >= 128 - R).astype(np.float16)
    L = (M <= -(128 - R)).astype(np.float16)
    s = np.float16(BSC * K2 * S31)
    return {"Im": (BSC * np.eye(128)).astype(np.float16),
            "In": (-np.eye(128)).astype(np.float16),
            "Dm": D, "Um": U, "Lm": L,
            "Ds": (D * s).astype(np.float16),
            "Us": (U * s).astype(np.float16),
            "Ls": (L * s).astype(np.float16)}


MATS_KEY = ("Im", "In", "Dm", "Um", "Lm", "Ds", "Us", "Ls")


def _build(nc):
    guide_d = nc.dram_tensor("guide", [3, H, W], F32, kind="ExternalInput").ap()
    input_d = nc.dram_tensor("input", [3, H, W], F32, kind="ExternalInput").ap()
    mats_all_d = nc.dram_tensor("Ms", [8, 128, 128], F16,
                                kind="ExternalInput").ap()
    out_d = nc.dram_tensor("out", [3, H, W], F32, kind="ExternalOutput").ap()

    def plane(dram, c):
        return dram[c].rearrange("(b p) w -> p b w", p=128)

    with tile.TileContext(nc) as tc, ExitStack() as ctx:
        cpool = ctx.enter_context(tc.tile_pool(name="consts", bufs=1))
        pads = ctx.enter_context(tc.tile_pool(name="pads", bufs=1))
        sop = ctx.enter_context(tc.tile_pool(name="sop", bufs=1))
        gt = ctx.enter_context(tc.tile_pool(name="gt", bufs=1))
        pers = ctx.enter_context(tc.tile_pool(name="pers", bufs=1))
        mpq = ctx.enter_context(tc.tile_pool(name="mpq", bufs=3))
        cvq = ctx.enter_context(tc.tile_pool(name="cvq", bufs=2))
        vwq = ctx.enter_context(tc.tile_pool(name="vwq", bufs=2))
        outp = ctx.enter_context(tc.tile_pool(name="outp", bufs=2))
        psum = ctx.enter_context(tc.tile_pool(name="psum", bufs=4, space="PSUM"))

        mats_t = cpool.tile([128, 8, 128], F16, tag="Ms", name="Ms")
        nc.sync.dma_start(mats_t[:], mats_all_d.rearrange("k p c -> p k c"))
        mats = {k: mats_t[:, i, :] for i, k in enumerate(MATS_KEY)}

        def padded(tag):
            return pads.tile([128, FULLW], F16, tag=tag, name=tag)

        def data_view(t):
            return t[:, 0:FLATW].rearrange("p (b w) -> p b w", b=NB)[
                :, :, PADL:PADL + W]

        def data_view3(t, c):
            return t[:, c, 0:FLATW].rearrange("p (b w) -> p b w", b=NB)[
                :, :, PADL:PADL + W]

        g16 = padded("g16")
        g2b = padded("g2b")
        pball = pads.tile([128, 3, FULLW], F16, tag="pball", name="pball")
        gpb = [padded(f"gp{c}") for c in range(3)]

        sot = [sop.tile([128, FLATW], F16, tag=f"so{i}", name=f"so{i}")
               for i in range(SO_N)]

        # ---- PE warm-up chain on Im (lands first in the mats blob) ----
        ps_warm = psum.tile([128, 2, W], F32, name="ps")
        for i in range(WARM_N):
            nc.tensor.matmul(ps_warm[:, 0, 0:128], mats["Im"], mats["Im"],
                             start=(i == 0), stop=(i == WARM_N - 1))

        # ---- casting input DMAs (SWDGE); Pool stream order matters ----
        gtiles = [gt.tile([128, NB, W], F16, tag=f"g{c}", name=f"g{c}")
                  for c in range(3)]
        nc.gpsimd.dma_start(data_view3(pball, 0), plane(input_d, 0))
        pb4 = pball[:, :, 0:FLATW].rearrange("p c (b w) -> p c b w", b=NB)
        nc.gpsimd.memset(pb4[:, :, :, 0:PADL], 0.0)
        nc.gpsimd.memset(pb4[:, :, :, PADL + W:PW], 0.0)
        nc.gpsimd.memset(pball[:, :, FLATW:FULLW], 0.0)
        for c in range(3):
            nc.gpsimd.dma_start(gtiles[c][:], plane(guide_d, c))
        nc.gpsimd.dma_start(data_view3(pball, 1), plane(input_d, 1))
        nc.gpsimd.dma_start(data_view3(pball, 2), plane(input_d, 2))
        gb4 = g16[:, 0:FLATW].rearrange("p (b w) -> p b w", b=NB)
        nc.gpsimd.memset(gb4[:, :, 0:PADL], 0.0)
        nc.gpsimd.memset(gb4[:, :, PADL + W:PW], 0.0)
        nc.gpsimd.memset(g16[:, FLATW:FULLW], 0.0)
        for t in (g2b, gpb[0], gpb[1], gpb[2]):
            nc.gpsimd.memset(t[:, FLATW:FULLW], 0.0)

        # ---- luma on DVE (TSP prescale at 4x + two 2x adds) ----
        for c, wc in ((0, WR), (1, WG), (2, WB)):
            nc.vector.tensor_scalar_mul(gtiles[c][:], gtiles[c][:], wc)
        nc.vector.tensor_add(gtiles[0][:], gtiles[0][:], gtiles[1][:])
        nc.vector.tensor_add(data_view(g16), gtiles[0][:], gtiles[2][:])

        # g^2 full-width on Act (pads stay zero)
        nc.scalar.square(g2b[:, 0:FLATW], g16[:, 0:FLATW])

        # ---- box building blocks ----
        so_ctr = [0]

        def wscan(src):
            so = sot[so_ctr[0] % SO_N]
            so_ctr[0] += 1
            nc.vector.tensor_tensor_scan(
                so[:], src[:, KW:KW + FLATW], src[:, 0:FLATW],
                0.0, ALU.add, ALU.subtract)
            return so

        def hpass(so, scaled=False, w_pre=None, post_sub=None):
            """two half-plane PSUM groups [128, 2, 512] (2 banks, 4 slots)"""
            Dm, Um, Lm = (mats["Ds"], mats["Us"], mats["Ls"]) if scaled else (
                mats["Dm"], mats["Um"], mats["Lm"])
            rvb = so[:].rearrange("p (b w) -> p b w", b=NB)

            def rv(j):
                return rvb[:, j, 16:16 + W]

            halves = []
            for h in range(2):
                ps = psum.tile([128, 2, W], F32, name="ps")
                for k in range(2):
                    i = 2 * h + k
                    mms = []
                    if w_pre is not None:
                        mms.append((mats["Im"], w_pre[:, i, :]))
                    mms.append((Dm, rv(i)))
                    if i + 1 < NB:
                        mms.append((Um, rv(i + 1)))
                    if i - 1 >= 0:
                        mms.append((Lm, rv(i - 1)))
                    if post_sub is not None:
                        mms.append((mats["In"], post_sub[:, i, :]))
                    for n, (lhsT, rhs) in enumerate(mms):
                        nc.tensor.matmul(ps[:, k, :], lhsT, rhs,
                                         start=(n == 0),
                                         stop=(n == len(mms) - 1))
                halves.append(ps)
            return halves

        def extract(dst, halves, fn):
            """dst: [128, NB, W]-shaped tile; apply fn per half"""
            for h, ps in enumerate(halves):
                fn(dst[:, 2 * h:2 * h + 2, :], ps[:])

        # ---- round 1 ----
        mp16 = [mpq.tile([128, NB, W], F16, tag="mp", name=f"mp{c}")
                for c in range(3)]

        def p_box(c):
            so_p = wscan(pball[:, c, :])
            hs = hpass(so_p)
            extract(mp16[c][:], hs, lambda d, s_: nc.scalar.mul(d, s_, S31))

        p_box(0)                                       # slot 0

        p_box(1)                                       # slot 1
        so_g = wscan(g16)                              # slot 2
        hs_g = hpass(so_g)
        mI = pers.tile([128, NB, W], F16, tag="mI", name="mI")
        extract(mI[:], hs_g, lambda d, s_: nc.scalar.mul(d, s_, S31))

        so_g2 = wscan(g2b)                             # slot 2
        hs_g2 = hpass(so_g2)

        sq = pers.tile([128, NB, W], F16, tag="sq", name="sq")
        nc.vector.tensor_mul(sq[:], mI[:], mI[:])
        vv = pers.tile([128, NB, W], F32, tag="vv", name="vv")
        for h, ps in enumerate(hs_g2):
            nc.vector.affine_then_add(vv[:, 2 * h:2 * h + 2, :],
                                      sq[:, 2 * h:2 * h + 2, :],
                                      ps[:], -1.0, E961)
        rr = pers.tile([128, NB, W], F32, tag="rr", name="rr")
        nc.vector.reciprocal_approx_fast(rr[:], vv[:])

        cvl = [cvq.tile([128, NB, W], F16, tag="cv", name=f"cv{c}")
               for c in range(3)]
        u2q = [cvq.tile([128, NB, W], F16, tag="u2", name=f"u2_{c}")
               for c in range(3)]

        def gp_box(c):
            # gp = g*p full-width on DVE (pads 0*0=0)
            nc.vector.tensor_mul(gpb[c][:, 0:FLATW], g16[:, 0:FLATW],
                                 pball[:, c, 0:FLATW])
            so_gp = wscan(gpb[c])
            nc.vector.tensor_mul(u2q[c][:], mI[:], mp16[c][:])
            hs = hpass(so_gp, post_sub=u2q[c])         # 961*cov in PSUM
            extract(cvl[c][:], hs, lambda d, s_: nc.scalar.copy(d, s_))
            return hs

        def ab_planes(c):
            a_v = data_view3(pball, c)
            nc.vector.tensor_mul(a_v, cvl[c][:], rr[:])      # a (rr fp32)
            vpr = vwq.tile([128, NB, W], F16, tag="vpr", name=f"vpr{c}")
            nc.gpsimd.tensor_mul(vpr[:], mI[:], a_v)         # a*mI (Pool)
            nc.gpsimd.tensor_sub(data_view(gpb[c]), mp16[c][:], vpr[:])

        ps_gp0 = gp_box(0)
        ab_planes(0)
        ps_gp1 = gp_box(1)                             # slot 5, psum 0
        p_box(2)                                       # slot 0, psum 1
        ab_planes(1)
        ps_gp2 = gp_box(2)                             # slot 1, psum 0
        ab_planes(2)

        # ---- round 2: scans first, then mm/extract phase ----
        soa = [None] * 3
        sob = [None] * 3
        for c in range(3):
            soa[c] = wscan(pball[:, c, :])
            sob[c] = wscan(gpb[c])

        wl = [None] * 3
        for c in range(3):
            hsa = hpass(soa[c])
            ea = vwq.tile([128, NB, W], F16, tag="ea", name=f"ea{c}")
            extract(ea[:], hsa, lambda d, s_: nc.scalar.mul(d, s_, K2))
            wl[c] = vwq.tile([128, NB, W], F16, tag="w", name=f"w{c}")
            nc.vector.tensor_mul(wl[c][:], ea[:], data_view(g16))

        for c in range(3):
            hsb = hpass(sob[c], scaled=True, w_pre=wl[c])
            o32 = outp.tile([128, NB, W], F32, tag="o", name=f"o{c}")
            extract(o32[:], hsb,
                    lambda d, s_: nc.scalar.mul(d, s_, 1.0 / BSC))
            nc.sync.dma_start(plane(out_d, c), o32[:])

    nc.compile()
    return nc


_NC_CACHE = None


def _get_nc():
    global _NC_CACHE
    if _NC_CACHE is None:
        nc = bacc.Bacc("TRN2", target_bir_lowering=False, debug=False)
        _build(nc)
        _NC_CACHE = nc
    return _NC_CACHE


def kernel(**inputs):
    guide = np.ascontiguousarray(inputs["guide"], dtype=np.float32)
    inp = np.ascontiguousarray(inputs["input"], dtype=np.float32)
    B = guide.shape[0]
    assert guide.shape == (8, 3, H, W) and inp.shape == (8, 3, H, W)
    mats = _band_blocks()
    ms = np.stack([mats[k] for k in MATS_KEY], axis=0)
    nc = _get_nc()
    in_maps = [
        {"guide": guide[i], "input": inp[i], "Ms": ms}
        for i in range(B)
    ]
    res = bass_utils.run_bass_kernel_spmd(nc, in_maps, core_ids=list(range(B)))
    return np.stack([res.results[i]["out"] for i in range(B)], axis=0)
